# revision 2
# baseline (speedup 1.0000x reference)
"""Trainium2 Bass kernel for BatchNorm2d + 8-head self-attention block.

Reference (per batch element b, all in fp32):
    xn = BN_eval(x[b])                          # per-channel affine
    t  = xn.T                                   # [S, C]
    q/k/v = t @ W.T + b                         # [S, 512] each, 8 heads x 64
    attn  = softmax(q k^T / 8)                  # per head, [S, S]
    y[b]  = (attn v) @ wo.T + bo, transposed    # [C, S]

Sharding: pure data-parallel — batch B=8, one batch element per NeuronCore,
weights replicated, no collectives.

Device-side design (per core), everything in the "transposed" domain so no
large transposes are ever needed:
  - BN is folded into the QKV weights/biases on the host; the 1/8 score
    scale is folded into wq/bq; v's bias is folded into bo.
  - Q^T,K^T [I,S] = wT.T @ x  (x arrives [C,S] — already the natural rhs)
  - V [S,I]      = x_chunk.T @ wvT  (x chunks are the stationary operand)
  - scores^T per head [t,s] = (K^T_h)^T.T @ Q^T_h; heads are row-packed in
    pairs on the PE via tile_position (0,0)/(64,0), K=64 each
  - softmax: scores are small (BN-normalized inputs), so exp without
    max-subtraction; denominators come FREE from the PV matmul via an
    extra ones-column on V (M=65); normalization is a per-head DVE multiply
    by the broadcast reciprocal row.
  - o^T [I,S] accumulates over 8 t-chunks (K=128); y^T = woT.T @ o^T + bo.
All matmuls run in fp32r (full PE rate at N=512, ~1e-4 relative error).
"""

import numpy as np

import concourse.bass as bass
import concourse.tile as tile
from concourse import bacc, mybir
from concourse.bass_utils import run_bass_kernel_spmd

B, C, S = 8, 512, 1024
H, DH, INNER = 8, 64, 512
EPS = 1e-5
SCALE = DH ** (-0.5)
N_CORES = 8
F32 = mybir.dt.float32
F32R = mybir.dt.float32r

_CACHE: dict = {}


def build_bass():
    nc = bacc.Bacc("TRN2", target_bir_lowering=False, debug=False,
                   num_devices=N_CORES)

    x_d = nc.dram_tensor("x", [C, S], F32, kind="ExternalInput")
    wqT_d = nc.dram_tensor("wqT", [C, INNER], F32, kind="ExternalInput")
    wkT_d = nc.dram_tensor("wkT", [C, INNER], F32, kind="ExternalInput")
    wvT_d = nc.dram_tensor("wvT", [C, INNER], F32, kind="ExternalInput")
    woT_d = nc.dram_tensor("woT", [INNER, C], F32, kind="ExternalInput")
    bq_d = nc.dram_tensor("bq", [INNER], F32, kind="ExternalInput")
    bk_d = nc.dram_tensor("bk", [INNER], F32, kind="ExternalInput")
    bo_d = nc.dram_tensor("bo", [C], F32, kind="ExternalInput")
    y_d = nc.dram_tensor("y", [C, S], F32, kind="ExternalOutput")

    KC = C // 128      # 4 contraction chunks over channels
    IT = INNER // 128  # 4 tiles over inner dim
    ST = S // 128      # 8 t-chunks
    NSLAB = S // 512   # 2 s-slabs

    with tile.TileContext(nc) as tc:
        with (
            tc.tile_pool(name="persist", bufs=1) as persist,
            tc.tile_pool(name="stage", bufs=3) as stage,
            tc.tile_pool(name="et", bufs=4) as etp,
            tc.tile_pool(name="norm", bufs=2) as normp,
            tc.tile_pool(name="psP", bufs=2, space="PSUM") as psP,
            tc.tile_pool(name="psS", bufs=2, space="PSUM") as psS,
            tc.tile_pool(name="psO", bufs=1, space="PSUM") as psO,
        ):
            # ---- load + cast x to fp32r ----
            xr = []
            for kc in range(KC):
                xst = stage.tile([128, S], F32, tag="xstage")
                nc.sync.dma_start(xst[:], x_d[kc * 128:(kc + 1) * 128, :])
                t = persist.tile([128, S], F32R, tag=f"xr{kc}")
                nc.vector.tensor_copy(t[:], xst[:])
                xr.append(t)

            # ---- load + cast weights to fp32r ----
            def load_w(dram):
                out = []
                for kc in range(KC):
                    wst = stage.tile([128, 512], F32, tag="wstage")
                    nc.sync.dma_start(wst[:], dram[kc * 128:(kc + 1) * 128, :])
                    t = persist.tile([128, 512], F32R, tag=f"w{dram.name}{kc}")
                    nc.vector.tensor_copy(t[:], wst[:])
                    out.append(t)
                return out

            wqT = load_w(wqT_d)
            wkT = load_w(wkT_d)
            wvT = load_w(wvT_d)
            woT = load_w(woT_d)

            # ---- biases as per-partition columns ----
            def load_b(dram, n):
                out = []
                for it in range(n):
                    t = persist.tile([128, 1], F32, tag=f"b{dram.name}{it}")
                    nc.sync.dma_start(t[:, 0:1], dram[it * 128:(it + 1) * 128, None])
                    out.append(t)
                return out

            bq = load_b(bq_d, IT)
            bk = load_b(bk_d, IT)
            bo = load_b(bo_d, IT)

            ones_sb = persist.tile([128, H], F32, tag="ones")
            nc.vector.memset(ones_sb[:], 1.0)

            # ---- Q^T, K^T = wT.T @ x + b   [IT][128, S] fp32r ----
            def project_T(wT, bias, name):
                outs = []
                for it in range(IT):
                    t = persist.tile([128, S], F32R, tag=f"{name}{it}")
                    outs.append(t)
                for it in range(IT):
                    for sl in range(NSLAB):
                        ps = psP.tile([128, 512], F32, tag="psP")
                        for kc in range(KC):
                            nc.tensor.matmul(
                                ps[:],
                                wT[kc][:, it * 128:(it + 1) * 128],
                                xr[kc][:, sl * 512:(sl + 1) * 512],
                                start=(kc == 0), stop=(kc == KC - 1),
                            )
                        nc.vector.tensor_scalar_add(
                            outs[it][:, sl * 512:(sl + 1) * 512], ps[:], bias[it][:]
                        )
                return outs

            qT = project_T(wqT, bq, "qT")
            kT = project_T(wkT, bk, "kT")

            # ---- V in interleaved [t-chunk][128, H*65] layout (+ones col) ----
            v_sb = []
            for tc_ in range(ST):
                t = persist.tile([128, H * 65], F32R, tag=f"v{tc_}")
                v_sb.append(t)
            for tc_ in range(ST):
                ps = psP.tile([128, 512], F32, tag="psP")
                for kc in range(KC):
                    nc.tensor.matmul(
                        ps[:],
                        xr[kc][:, tc_ * 128:(tc_ + 1) * 128],
                        wvT[kc][:],
                        start=(kc == 0), stop=(kc == KC - 1),
                    )
                vv = v_sb[tc_][:].rearrange("p (h m) -> p h m", h=H)
                nc.vector.tensor_copy(
                    vv[:, :, 0:64], ps[:].rearrange("p (h m) -> p h m", h=H)
                )
                nc.vector.tensor_copy(vv[:, :, 64:65], ones_sb[:, :, None])

            # ---- attention: head pairs, transposed domain ----
            oT = [
                persist.tile([128, S], F32R, tag=f"oT{i}", name=f"oT{i}")
                for i in range(IT)
            ]
            for hp in range(H // 2):
                for sl in range(NSLAB):
                    po0 = psO.tile([65, 512], F32, tag="po0")
                    po1 = psO.tile([65, 512], F32, tag="po1")
                    for tc_ in range(ST):
                        pss = psS.tile([128, 1024], F32, tag="psS")
                        # scores^T for the head pair, row-packed K=64
                        nc.tensor.matmul(
                            pss[:, 0:512],
                            kT[hp][0:64, tc_ * 128:(tc_ + 1) * 128],
                            qT[hp][0:64, sl * 512:(sl + 1) * 512],
                            start=True, stop=True, tile_position=(0, 0),
                        )
                        nc.tensor.matmul(
                            pss[:, 512:1024],
                            kT[hp][64:128, tc_ * 128:(tc_ + 1) * 128],
                            qT[hp][64:128, sl * 512:(sl + 1) * 512],
                            start=True, stop=True, tile_position=(64, 0),
                        )
                        # exp of both heads' chunk in one ACT call
                        et = etp.tile([128, 1024], F32R, tag="et")
                        nc.scalar.activation(
                            et[:], pss[:], mybir.ActivationFunctionType.Exp
                        )
                        # PV accumulation (+ones row -> denominators)
                        h0, h1 = 2 * hp, 2 * hp + 1
                        nc.tensor.matmul(
                            po0[:],
                            v_sb[tc_][:, h0 * 65:(h0 + 1) * 65],
                            et[:, 0:512],
                            start=(tc_ == 0), stop=(tc_ == ST - 1),
                        )
                        nc.tensor.matmul(
                            po1[:],
                            v_sb[tc_][:, h1 * 65:(h1 + 1) * 65],
                            et[:, 512:1024],
                            start=(tc_ == 0), stop=(tc_ == ST - 1),
                        )
                    # normalize each head: oT rows = po[0:64] * (1/denom row)
                    for half, po in ((0, po0), (1, po1)):
                        rrow = normp.tile([1, 512], F32, tag="rrow")
                        nc.vector.reciprocal(rrow[:], po[64:65, :])
                        rbc = normp.tile([64, 512], F32, tag="rbc")
                        nc.gpsimd.partition_broadcast(rbc[:], rrow[:])
                        nc.vector.tensor_mul(
                            oT[hp][half * 64:(half + 1) * 64,
                                   sl * 512:(sl + 1) * 512],
                            po[0:64, :],
                            rbc[:],
                        )

            # ---- y^T = woT.T @ o^T + bo ----
            for ct in range(IT):
                for sl in range(NSLAB):
                    ps = psP.tile([128, 512], F32, tag="psP")
                    for ic in range(IT):
                        nc.tensor.matmul(
                            ps[:],
                            woT[ic][:, ct * 128:(ct + 1) * 128],
                            oT[ic][:, sl * 512:(sl + 1) * 512],
                            start=(ic == 0), stop=(ic == IT - 1),
                        )
                    ysb = stage.tile([128, 512], F32, tag="ysb")
                    nc.vector.tensor_scalar_add(ysb[:], ps[:], bo[ct][:])
                    nc.sync.dma_start(
                        y_d[ct * 128:(ct + 1) * 128, sl * 512:(sl + 1) * 512],
                        ysb[:],
                    )

    nc.compile()
    return nc


def prep_host(inputs):
    """Fold BN + scale + v-bias into effective weights (fp32 numpy)."""
    x = np.asarray(inputs["x"], dtype=np.float32)
    g = np.asarray(inputs["bn_gamma"], dtype=np.float32)
    be = np.asarray(inputs["bn_beta"], dtype=np.float32)
    mu = np.asarray(inputs["bn_mean"], dtype=np.float32)
    var = np.asarray(inputs["bn_var"], dtype=np.float32)
    wq = np.asarray(inputs["wq"], dtype=np.float32)
    bq = np.asarray(inputs["bq"], dtype=np.float32)
    wk = np.asarray(inputs["wk"], dtype=np.float32)
    bk = np.asarray(inputs["bk"], dtype=np.float32)
    wv = np.asarray(inputs["wv"], dtype=np.float32)
    bv = np.asarray(inputs["bv"], dtype=np.float32)
    wo = np.asarray(inputs["wo"], dtype=np.float32)
    bo = np.asarray(inputs["bo"], dtype=np.float32)

    a = g / np.sqrt(var + EPS)          # [C]
    bvec = be - mu * a                  # [C]

    wq_eff = wq * a[None, :] * SCALE
    bq_eff = (bq + wq @ bvec) * SCALE
    wk_eff = wk * a[None, :]
    bk_eff = bk + wk @ bvec
    wv_eff = wv * a[None, :]
    bv_eff = bv + wv @ bvec
    bo_eff = bo + wo @ bv_eff           # v bias rides through softmax (sums to 1)

    per_core = []
    for b in range(B):
        per_core.append({
            "x": np.ascontiguousarray(x[b, :, :, 0]),
            "wqT": np.ascontiguousarray(wq_eff.T),
            "wkT": np.ascontiguousarray(wk_eff.T),
            "wvT": np.ascontiguousarray(wv_eff.T),
            "woT": np.ascontiguousarray(wo.T),
            "bq": bq_eff,
            "bk": bk_eff,
            "bo": bo_eff,
        })
    return per_core


def kernel(**inputs):
    if "nc" not in _CACHE:
        _CACHE["nc"] = build_bass()
    nc = _CACHE["nc"]
    in_maps = prep_host(inputs)
    res = run_bass_kernel_spmd(nc, in_maps, list(range(N_CORES)))
    y = np.stack([res.results[c]["y"] for c in range(N_CORES)], axis=0)
    return y[..., None].astype(np.float32)


def run_traced(**inputs):
    """Like kernel() but with NTFF profiling; returns (y, BassKernelResults)."""
    if "nc" not in _CACHE:
        _CACHE["nc"] = build_bass()
    nc = _CACHE["nc"]
    in_maps = prep_host(inputs)
    import tempfile
    tmpdir = tempfile.mkdtemp(prefix="mha_trace_")
    res = run_bass_kernel_spmd(
        nc, in_maps, list(range(N_CORES)), trace=True, tmpdir=tmpdir
    )
    y = np.stack([res.results[c]["y"] for c in range(N_CORES)], axis=0)
    return y[..., None].astype(np.float32), res, tmpdir


# revision 5
# speedup vs baseline: 1.2689x; 1.2689x over previous
"""Trainium2 Bass kernel for BatchNorm2d + 8-head self-attention block.

Reference (per batch element b, all fp32):
    xn = BN_eval(x[b]); t = xn.T
    q/k/v = t @ W.T + b            # [S, 512], 8 heads x 64
    attn  = softmax(q k^T / 8)     # per head
    y[b]  = ((attn v) @ wo.T + bo).T

Sharding: pure data parallel — one batch element per NeuronCore, weights
replicated, no collectives.

Device design (per core), fully in the "transposed" domain (no large
transposes anywhere):
  - BN folded into QKV weights/biases on host; 1/8 scale folded into wq/bq;
    v bias folded into bo (softmax rows sum to 1).
  - Q^T,K^T [I,S] = wT.T @ x      (x arrives [C,S] — natural rhs)
  - V [S,I]       = x_chunk.T @ wvT, stored interleaved per head with a
    ones column ([128, 8*65]) so the PV matmul (M=65) also produces the
    softmax denominators for free.
  - scores^T per head [t,s]; head pairs row-packed via tile_position
    (0,0)/(64,0), K=64 each; exp on ScalarE over both heads in one call
    (no max subtraction — BN-normalized inputs keep scores small).
  - o^T accumulates over 8 t-chunks (K=128); normalize = approx-reciprocal
    row + gpsimd partition-broadcast + DVE multiply; y^T = woT.T @ o^T + bo.
All matmuls in fp32r (full PE rate at N=512, ~1e-4 relative error).
"""

import numpy as np

import concourse.bass as bass
import concourse.tile as tile
from concourse import bacc, mybir
from concourse.bass_utils import run_bass_kernel_spmd

B, C, S = 8, 512, 1024
H, DH, INNER = 8, 64, 512
EPS = 1e-5
SCALE = DH ** (-0.5)
N_CORES = 8
F32 = mybir.dt.float32
F32R = mybir.dt.float32r

_CACHE: dict = {}

KC = C // 128      # 4 contraction chunks over channels
IT = INNER // 128  # 4 tiles over inner dim (also head-pair index)
ST = S // 128      # 8 t-chunks
NSLAB = S // 512   # 2 s-slabs


def build_bass():
    nc = bacc.Bacc("TRN2", target_bir_lowering=False, debug=False,
                   num_devices=N_CORES)

    x_d = nc.dram_tensor("x", [C, S], F32, kind="ExternalInput")
    wqT_d = nc.dram_tensor("wqT", [C, INNER], F32, kind="ExternalInput")
    wkT_d = nc.dram_tensor("wkT", [C, INNER], F32, kind="ExternalInput")
    wvT_d = nc.dram_tensor("wvT", [C, INNER], F32, kind="ExternalInput")
    woT_d = nc.dram_tensor("woT", [INNER, C], F32, kind="ExternalInput")
    bq_d = nc.dram_tensor("bq", [INNER], F32, kind="ExternalInput")
    bk_d = nc.dram_tensor("bk", [INNER], F32, kind="ExternalInput")
    bo_d = nc.dram_tensor("bo", [C], F32, kind="ExternalInput")
    y_d = nc.dram_tensor("y", [C, S], F32, kind="ExternalOutput")

    with tile.TileContext(nc) as tc:
        with (
            tc.tile_pool(name="persist", bufs=1) as persist,
            tc.tile_pool(name="stage", bufs=2) as stage,
            tc.tile_pool(name="out", bufs=3) as outp,
            tc.tile_pool(name="et", bufs=4) as etp,
            tc.tile_pool(name="norm", bufs=2) as normp,
            tc.tile_pool(name="psP", bufs=2, space="PSUM") as psP,
            tc.tile_pool(name="psS", bufs=2, space="PSUM") as psS,
            tc.tile_pool(name="psO", bufs=1, space="PSUM") as psO,
        ):
            # ---- loads: one big DMA per tensor, split across both HWDGE
            # queues (sync + scalar) so x and the first weights land fast ----
            x_st = stage.tile([128, KC, S], F32, tag="x_st", bufs=1)
            x_r3 = x_d.rearrange("(k p) s -> p k s", p=128)
            nc.sync.dma_start(x_st[:, 0:2, :], x_r3[:, 0:2, :])
            wq_st = stage.tile([128, KC, 512], F32, tag="wq_st", bufs=1)
            nc.scalar.dma_start(wq_st[:], wqT_d.rearrange("(k p) i -> p k i", p=128))
            nc.sync.dma_start(x_st[:, 2:4, :], x_r3[:, 2:4, :])
            wk_st = stage.tile([128, KC, 512], F32, tag="wk_st", bufs=1)
            nc.scalar.dma_start(wk_st[:], wkT_d.rearrange("(k p) i -> p k i", p=128))
            wv_st = stage.tile([128, KC, 512], F32, tag="wv_st", bufs=1)
            nc.sync.dma_start(wv_st[:], wvT_d.rearrange("(k p) i -> p k i", p=128))
            wo_st = stage.tile([128, KC, 512], F32, tag="wo_st", bufs=1)
            nc.scalar.dma_start(wo_st[:], woT_d.rearrange("(k p) i -> p k i", p=128))

            bq_sb = persist.tile([128, IT], F32, tag="bq")
            nc.scalar.dma_start(bq_sb[:], bq_d.rearrange("(t p) -> p t", p=128))
            bk_sb = persist.tile([128, IT], F32, tag="bk")
            nc.scalar.dma_start(bk_sb[:], bk_d.rearrange("(t p) -> p t", p=128))
            bo_sb = persist.tile([128, IT], F32, tag="bo")
            nc.scalar.dma_start(bo_sb[:], bo_d.rearrange("(t p) -> p t", p=128))

            # ---- casts to fp32r ----
            xr = persist.tile([128, KC, S], F32R, tag="xr", name="xr")
            for kc in range(KC):
                nc.vector.tensor_copy(xr[:, kc, :], x_st[:, kc, :])
            wqr = persist.tile([128, KC, 512], F32R, tag="wqr", name="wqr")
            nc.vector.tensor_copy(wqr[:], wq_st[:])
            wkr = persist.tile([128, KC, 512], F32R, tag="wkr", name="wkr")
            nc.vector.tensor_copy(wkr[:], wk_st[:])
            wvr = persist.tile([128, KC, 512], F32R, tag="wvr", name="wvr")
            nc.vector.tensor_copy(wvr[:], wv_st[:])
            wor = persist.tile([128, KC, 512], F32R, tag="wor", name="wor")
            nc.vector.tensor_copy(wor[:], wo_st[:])

            ones_sb = persist.tile([128, H], F32, tag="ones")
            nc.vector.memset(ones_sb[:], 1.0)

            # ---- persistent per-slab outputs ----
            qT = [[persist.tile([128, 512], F32R, tag=f"qT{i}{s}",
                                name=f"qT{i}{s}") for s in range(NSLAB)]
                  for i in range(IT)]
            kT = [[persist.tile([128, 512], F32R, tag=f"kT{i}{s}",
                                name=f"kT{i}{s}") for s in range(NSLAB)]
                  for i in range(IT)]
            oT = [[persist.tile([128, 512], F32R, tag=f"oT{i}{s}",
                                name=f"oT{i}{s}") for s in range(NSLAB)]
                  for i in range(IT)]
            v_sb = [persist.tile([128, H * 65], F32R, tag=f"v{t}",
                                 name=f"v{t}") for t in range(ST)]

            def qk_proj(hp):
                for w, bias, dst in ((wqr, bq_sb, qT), (wkr, bk_sb, kT)):
                    for sl in range(NSLAB):
                        ps = psP.tile([128, 512], F32, tag="psP", name="psP")
                        for kc in range(KC):
                            nc.tensor.matmul(
                                ps[:],
                                w[:, kc, hp * 128:(hp + 1) * 128],
                                xr[:, kc, sl * 512:(sl + 1) * 512],
                                start=(kc == 0), stop=(kc == KC - 1),
                            )
                        nc.vector.tensor_scalar_add(
                            dst[hp][sl][:], ps[:], bias[:, hp:hp + 1]
                        )

            def v_proj(tc_):
                ps = psP.tile([128, 512], F32, tag="psP", name="psP")
                for kc in range(KC):
                    nc.tensor.matmul(
                        ps[:],
                        xr[:, kc, tc_ * 128:(tc_ + 1) * 128],
                        wvr[:, kc, :],
                        start=(kc == 0), stop=(kc == KC - 1),
                    )
                vv = v_sb[tc_][:].rearrange("p (h m) -> p h m", h=H)
                nc.vector.tensor_copy(
                    vv[:, :, 0:64], ps[:].rearrange("p (h m) -> p h m", h=H)
                )
                nc.vector.tensor_copy(vv[:, :, 64:65], ones_sb[:, :, None])

            def attention(sl, hp):
                h0, h1 = 2 * hp, 2 * hp + 1
                po0 = psO.tile([65, 512], F32, tag="po0", name="po0")
                po1 = psO.tile([65, 512], F32, tag="po1", name="po1")
                for tc_ in range(ST):
                    ksl, kcol = tc_ // 4, (tc_ % 4) * 128
                    pss = psS.tile([128, 1024], F32, tag="psS", name="psS")
                    nc.tensor.matmul(
                        pss[:, 0:512],
                        kT[hp][ksl][0:64, kcol:kcol + 128],
                        qT[hp][sl][0:64, :],
                        start=True, stop=True, tile_position=(0, 0),
                    )
                    nc.tensor.matmul(
                        pss[:, 512:1024],
                        kT[hp][ksl][64:128, kcol:kcol + 128],
                        qT[hp][sl][64:128, :],
                        start=True, stop=True, tile_position=(64, 0),
                    )
                    et = etp.tile([128, 1024], F32R, tag="et", name="et")
                    nc.scalar.activation(
                        et[:], pss[:], mybir.ActivationFunctionType.Exp
                    )
                    nc.tensor.matmul(
                        po0[:], v_sb[tc_][:, h0 * 65:(h0 + 1) * 65],
                        et[:, 0:512],
                        start=(tc_ == 0), stop=(tc_ == ST - 1),
                    )
                    nc.tensor.matmul(
                        po1[:], v_sb[tc_][:, h1 * 65:(h1 + 1) * 65],
                        et[:, 512:1024],
                        start=(tc_ == 0), stop=(tc_ == ST - 1),
                    )
                for half, po in ((0, po0), (1, po1)):
                    drow = normp.tile([1, 512], F32, tag="drow", name="drow")
                    nc.vector.tensor_copy(drow[:], po[64:65, :])
                    rrow = normp.tile([1, 512], F32, tag="rrow", name="rrow")
                    nc.vector.reciprocal_approx_fast(rrow[:], drow[:])
                    rbc = normp.tile([64, 512], F32, tag="rbc", name="rbc")
                    nc.gpsimd.partition_broadcast(rbc[:], rrow[:])
                    nc.vector.tensor_mul(
                        oT[hp][sl][half * 64:(half + 1) * 64, :],
                        po[0:64, :],
                        rbc[:],
                    )

            def out_proj(sl):
                for ct in range(IT):
                    ps = psP.tile([128, 512], F32, tag="psP", name="psP")
                    for ic in range(IT):
                        nc.tensor.matmul(
                            ps[:],
                            wor[:, ic, ct * 128:(ct + 1) * 128],
                            oT[ic][sl][:],
                            start=(ic == 0), stop=(ic == IT - 1),
                        )
                    ysb = outp.tile([128, 512], F32, tag="ysb", name="ysb")
                    nc.vector.tensor_scalar_add(ysb[:], ps[:], bo_sb[:, ct:ct + 1])
                    nc.sync.dma_start(
                        y_d[ct * 128:(ct + 1) * 128, sl * 512:(sl + 1) * 512],
                        ysb[:],
                    )

            # ---- emission order (priority hint for the scheduler) ----
            qk_proj(0)
            for tc_ in range(ST):
                v_proj(tc_)
            emitted = {0}
            for sl in range(NSLAB):
                for hp in range(IT):
                    if hp not in emitted:
                        qk_proj(hp)
                        emitted.add(hp)
                    attention(sl, hp)
                out_proj(sl)

    nc.compile()
    return nc


def prep_host(inputs):
    """Fold BN + scale + v-bias into effective weights (fp32 numpy)."""
    x = np.asarray(inputs["x"], dtype=np.float32)
    g = np.asarray(inputs["bn_gamma"], dtype=np.float32)
    be = np.asarray(inputs["bn_beta"], dtype=np.float32)
    mu = np.asarray(inputs["bn_mean"], dtype=np.float32)
    var = np.asarray(inputs["bn_var"], dtype=np.float32)
    wq = np.asarray(inputs["wq"], dtype=np.float32)
    bq = np.asarray(inputs["bq"], dtype=np.float32)
    wk = np.asarray(inputs["wk"], dtype=np.float32)
    bk = np.asarray(inputs["bk"], dtype=np.float32)
    wv = np.asarray(inputs["wv"], dtype=np.float32)
    bv = np.asarray(inputs["bv"], dtype=np.float32)
    wo = np.asarray(inputs["wo"], dtype=np.float32)
    bo = np.asarray(inputs["bo"], dtype=np.float32)

    a = g / np.sqrt(var + EPS)          # [C]
    bvec = be - mu * a                  # [C]

    wq_eff = wq * a[None, :] * SCALE
    bq_eff = (bq + wq @ bvec) * SCALE
    wk_eff = wk * a[None, :]
    bk_eff = bk + wk @ bvec
    wv_eff = wv * a[None, :]
    bv_eff = bv + wv @ bvec
    bo_eff = bo + wo @ bv_eff           # v bias rides through softmax (sums to 1)

    per_core = []
    for b in range(B):
        per_core.append({
            "x": np.ascontiguousarray(x[b, :, :, 0]),
            "wqT": np.ascontiguousarray(wq_eff.T),
            "wkT": np.ascontiguousarray(wk_eff.T),
            "wvT": np.ascontiguousarray(wv_eff.T),
            "woT": np.ascontiguousarray(wo.T),
            "bq": bq_eff,
            "bk": bk_eff,
            "bo": bo_eff,
        })
    return per_core


def kernel(**inputs):
    if "nc" not in _CACHE:
        _CACHE["nc"] = build_bass()
    nc = _CACHE["nc"]
    in_maps = prep_host(inputs)
    res = run_bass_kernel_spmd(nc, in_maps, list(range(N_CORES)))
    y = np.stack([res.results[c]["y"] for c in range(N_CORES)], axis=0)
    return y[..., None].astype(np.float32)


def run_traced(**inputs):
    """Like kernel() but with NTFF profiling; returns (y, results, tmpdir)."""
    if "nc" not in _CACHE:
        _CACHE["nc"] = build_bass()
    nc = _CACHE["nc"]
    in_maps = prep_host(inputs)
    import tempfile
    tmpdir = tempfile.mkdtemp(prefix="mha_trace_")
    res = run_bass_kernel_spmd(
        nc, in_maps, list(range(N_CORES)), trace=True, tmpdir=tmpdir
    )
    y = np.stack([res.results[c]["y"] for c in range(N_CORES)], axis=0)
    return y[..., None].astype(np.float32), res, tmpdir
